# revision 17
# baseline (speedup 1.0000x reference)
"""Trainium2 Bass kernel for MemoryEfficientAttention.

Model: out = softmax((x@Wq)(x@Wk)^T / sqrt(dk)) (x@Wv) @ W_out
  x [2, 4096, 512], W_qkv [512, 1536], W_out [512, 512], H=8, dk=64.

Distribution across 8 NeuronCores (no collectives):
  device d handles batch b = d//4 and query rows [(d%4)*1024, +1024).
  Each device computes k/v projections for its full batch (4x redundant),
  q projection for its row slice, attention in the transposed (S^T)
  orientation so the P@v matmul needs no transposes, and its slice of the
  output projection, emitted transposed [512, 1024]; the host stitches the
  full [2, 4096, 512] output back together.

Numerics: matmuls run as float32r (TF32-like, ~1.5e-4 rel err, full PE
speed at N>=256); softmax skips the max-subtraction (scores are O(1) by
construction) and folds the 1/sqrt(dk) scale into the Exp activation; the
softmax denominator comes from a ones-column appended to v.
"""

import sys

for _p in ("/opt/trn_rl_repo",):
    if _p not in sys.path:
        sys.path.insert(0, _p)

import json
from contextlib import ExitStack

import numpy as np

import concourse.bass as bass
import concourse.bass2jax as _b2j
import concourse.bass_utils as _bu
import concourse.mybir as mybir
import concourse.tile as tile

# ---------------------------------------------------------------------------
# Workaround: this walrus build rejects >1 sync wait per instruction. Split
# excess on_wait entries onto injected single-wait EventSemaphore
# instructions on the same engine right before the original instruction.
# ---------------------------------------------------------------------------
_orig_compile_bir_kernel = _bu.compile_bir_kernel


def _split_excess_waits(bir_bytes):
    bir = json.loads(bir_bytes)
    n = 0
    for fn in bir.get("functions", []):
        for blk in fn.get("blocks", []):
            out = []
            for ins in blk.get("instructions", []):
                si = ins.get("sync_info")
                if si:
                    ow = si.get("on_wait") or []
                    if len(ow) > 1:
                        for w in ow[:-1]:
                            n += 1
                            out.append({
                                "debug": ins.get("debug", 0),
                                "engine": ins["engine"],
                                "ins": [],
                                "outs": [],
                                "name": f"{ins['name']}-xw{n}",
                                "opcode": "EventSemaphore",
                                "sync_info": {"on_update": [], "on_wait": [w]},
                            })
                        si["on_wait"] = [ow[-1]]
                out.append(ins)
            blk["instructions"] = out
    return json.dumps(bir).encode()


def _patched_compile_bir_kernel(bir_json, tmpdir, neff_name="file.neff"):
    if isinstance(bir_json, str):
        bir_json = bir_json.encode()
    return _orig_compile_bir_kernel(_split_excess_waits(bir_json), tmpdir, neff_name)


if getattr(_bu.compile_bir_kernel, "__name__", "") != "_patched_compile_bir_kernel":
    _bu.compile_bir_kernel = _patched_compile_bir_kernel
    _b2j.compile_bir_kernel = _patched_compile_bir_kernel

# ---------------------------------------------------------------------------
# Problem constants (hardcoded per the harness contract)
# ---------------------------------------------------------------------------
B, S, D = 2, 4096, 512
H, DK = 8, 64
NDEV = 8
ROWS = (B * S) // NDEV          # 1024 query rows per device
DEV_PER_BATCH = NDEV // B       # 4
NT = S // 128                   # 32 key tiles
NTB = S // 512                  # 8 projection t-blocks
NPAIR = H // 2                  # 4 head pairs
VW = 65                         # v width per head incl. ones column

f32 = mybir.dt.float32
f32r = mybir.dt.float32r
EXPF = mybir.ActivationFunctionType.Exp


def _build_nc(rep=1):
    nc = bass.Bass()
    xT = nc.dram_tensor("xT", [D, S], f32, kind="ExternalInput")
    wqkv = nc.dram_tensor("wqkv", [D, 3 * D], f32, kind="ExternalInput")
    wout = nc.dram_tensor("wout", [D, D], f32, kind="ExternalInput")
    xTq = nc.dram_tensor("xTq", [D, ROWS], f32, kind="ExternalInput")
    outT = nc.dram_tensor("outT", [D, ROWS], f32, kind="ExternalOutput")
    bf16 = mybir.dt.bfloat16

    with tile.TileContext(nc) as tc:
        with tc.tile_pool(name="kvq", bufs=1) as kvq, \
             tc.tile_pool(name="np_", bufs=1) as np_, \
             tc.tile_pool(name="xs", bufs=2) as xs, \
             tc.tile_pool(name="ptp", bufs=3) as ptp, \
             tc.tile_pool(name="zp", bufs=1) as zp, \
             tc.tile_pool(name="osp", bufs=2) as osp, \
             tc.tile_pool(name="dram", bufs=1, space="DRAM") as dramp:

            kT = [kvq.tile([128, S], f32r, tag=f"kT{c}", name=f"kT{c}")
                  for c in range(4)]
            qT = [kvq.tile([128, ROWS], f32r, tag=f"qT{c}", name=f"qT{c}")
                  for c in range(4)]
            vsb = kvq.tile([128, NT * H * VW], bf16, tag="vsb", name="vsb")
            uctx = [np_.tile([128, ROWS], f32r, tag=f"uctx{p}", name=f"uctx{p}")
                    for p in range(NPAIR)]
            wk = np_.tile([128, 4 * 512], f32r, tag="wk", name="wk")
            wv = np_.tile([128, 4 * 512], f32r, tag="wv", name="wv")
            wqq = np_.tile([128, 4 * 512], f32r, tag="wqq", name="wqq")
            wot = np_.tile([128, 4 * D], f32r, tag="wot", name="wot")
            dz = dramp.tile([1, H * ROWS], f32, name="dz")

            for wtile, lo in ((wqq, 0), (wk, D), (wv, 2 * D)):
                nc.sync.dma_start(
                    wtile[:].rearrange("p (c f) -> p c f", c=4),
                    wqkv[:, lo:lo + D].rearrange("(c p) f -> p c f", p=128)
                    .bitcast(f32r))
            nc.sync.dma_start(
                wot[:].rearrange("p (c f) -> p c f", c=4),
                wout[:, :].rearrange("(c p) f -> p c f", p=128).bitcast(f32r))

            # ones columns of v (staged via the rzb slot, reused later)
            ones8 = zp.tile([128, 512], f32, tag="rzb", name="ones8")
            nc.gpsimd.memset(ones8[:], 1.0)
            nc.vector.tensor_copy(
                vsb[:].rearrange("p (c w) -> p c w", w=VW)[:, :, DK:DK + 1],
                ones8[:, 0:NT * H].rearrange("p (c f) -> p c f", f=1))

            rep_ctx = tc.For_i(0, rep, 1) if rep > 1 else None
            if rep_ctx is not None:
                rep_ctx.__enter__()

            # ---------- prologue: all projections, one x stream ----------
            with tc.tile_pool(name="ps_pr", bufs=3, space="PSUM") as ps_pr:
                for tb in range(NTB):
                    xc = xs.tile([128, 4 * 512], f32r, tag="xc", name="xc")
                    nc.sync.dma_start(
                        xc[:].rearrange("p (c f) -> p c f", c=4),
                        xT[:, tb * 512:(tb + 1) * 512]
                        .rearrange("(c p) f -> p c f", p=128).bitcast(f32r))
                    for c in range(4):
                        pk = ps_pr.tile([128, 512], f32, tag="pp", name="pk")
                        for i in range(4):
                            nc.tensor.matmul(
                                pk[:],
                                wk[:, i * 512 + c * 128:i * 512 + (c + 1) * 128],
                                xc[:, i * 512:(i + 1) * 512],
                                start=(i == 0), stop=(i == 3))
                        nc.vector.tensor_copy(
                            kT[c][:, tb * 512:(tb + 1) * 512], pk[:])
                    for tt in range(4):
                        gt = tb * 4 + tt
                        pv = ps_pr.tile([128, 512], f32, tag="pp", name="pv")
                        for i in range(4):
                            nc.tensor.matmul(
                                pv[:],
                                xc[:, i * 512 + tt * 128:i * 512 + (tt + 1) * 128],
                                wv[:, i * 512:(i + 1) * 512],
                                start=(i == 0), stop=(i == 3))
                        vdst = vsb[:, gt * H * VW:(gt + 1) * H * VW].rearrange(
                            "p (h w) -> p h w", w=VW)[:, :, 0:DK]
                        nc.vector.tensor_copy(
                            vdst, pv[:].rearrange("p (h e) -> p h e", e=DK))
                for seg in range(2):
                    xq = xs.tile([128, 4 * 512], f32r, tag="xc", name="xq")
                    nc.sync.dma_start(
                        xq[:].rearrange("p (c f) -> p c f", c=4),
                        xTq[:, seg * 512:(seg + 1) * 512]
                        .rearrange("(c p) f -> p c f", p=128).bitcast(f32r))
                    for c in range(4):
                        pq = ps_pr.tile([128, 512], f32, tag="pp", name="pq")
                        for i in range(4):
                            nc.tensor.matmul(
                                pq[:],
                                wqq[:, i * 512 + c * 128:i * 512 + (c + 1) * 128],
                                xq[:, i * 512:(i + 1) * 512],
                                start=(i == 0), stop=(i == 3))
                        nc.vector.tensor_copy(
                            qT[c][:, seg * 512:(seg + 1) * 512], pq[:])

            # ---------- attention: 3-seg PSUM ring, 2048-wide exp ----------
            with tc.tile_pool(name="ps_r", bufs=1, space="PSUM") as ps_r, \
                 tc.tile_pool(name="ps_c", bufs=1, space="PSUM") as ps_c:
                for p in range(NPAIR):
                    for sb in range(2):
                        s0 = sb * 512
                        ring = ps_r.tile([128, 3072], f32, tag="ring", name="ring")
                        c0 = ps_c.tile([VW, 512], f32, tag="c0", name="c0")
                        c1 = ps_c.tile([VW, 512], f32, tag="c1", name="c1")
                        pts = {}
                        for t in range(NT):
                            sg = (t % 3) * 1024
                            nc.tensor.matmul(
                                ring[:, sg:sg + 512],
                                kT[p][0:64, t * 128:(t + 1) * 128],
                                qT[p][0:64, s0:s0 + 512],
                                start=True, stop=True)
                            nc.tensor.matmul(
                                ring[:, sg + 512:sg + 1024],
                                kT[p][64:128, t * 128:(t + 1) * 128],
                                qT[p][64:128, s0:s0 + 512],
                                start=True, stop=True)
                            if t % 2 == 1:
                                a, b = (t - 1) % 3, t % 3
                                pt = ptp.tile([128, 2048], bf16, tag="pt",
                                              name="pt")
                                if b == a + 1:
                                    nc.scalar.activation(
                                        pt[:], ring[:, a * 1024:a * 1024 + 2048],
                                        EXPF, scale=0.125)
                                else:  # wrap (a=2, b=0): two ops
                                    nc.scalar.activation(
                                        pt[:, 0:1024],
                                        ring[:, a * 1024:a * 1024 + 1024],
                                        EXPF, scale=0.125)
                                    nc.scalar.activation(
                                        pt[:, 1024:2048],
                                        ring[:, b * 1024:b * 1024 + 1024],
                                        EXPF, scale=0.125)
                                pts[t - 1] = (pt, 0)
                                pts[t] = (pt, 1024)
                                for tt in (t - 1, t):
                                    ptt, off = pts.pop(tt)
                                    nc.tensor.matmul(
                                        c0[:],
                                        vsb[:, (tt * H + 2 * p) * VW:
                                             (tt * H + 2 * p) * VW + VW],
                                        ptt[:, off:off + 512],
                                        start=(tt == 0), stop=(tt == NT - 1))
                                    nc.tensor.matmul(
                                        c1[:],
                                        vsb[:, (tt * H + 2 * p + 1) * VW:
                                             (tt * H + 2 * p + 1) * VW + VW],
                                        ptt[:, off + 512:off + 1024],
                                        start=(tt == 0), stop=(tt == NT - 1))
                        # fast PSUM release: unnormalized ctx + Z to SBUF
                        zr = zp.tile([64, 1024], f32, tag="zr", name="zr")
                        nc.vector.tensor_copy(zr[0:1, 0:512], c0[64:65, :])
                        nc.vector.tensor_copy(zr[32:33, 0:512], c1[64:65, :])
                        nc.vector.tensor_copy(
                            uctx[p][0:64, s0:s0 + 512], c0[0:64, :])
                        nc.vector.tensor_copy(
                            uctx[p][64:128, s0:s0 + 512], c1[0:64, :])
                        # normalize off the critical path (DVE + DMA only)
                        nc.vector.reciprocal(zr[:, 512:1024], zr[:, 0:512])
                        o0 = 2 * p * ROWS + s0
                        o1 = (2 * p + 1) * ROWS + s0
                        nc.sync.dma_start(dz[0:1, o0:o0 + 512],
                                          zr[0:1, 512:1024])
                        nc.sync.dma_start(dz[0:1, o1:o1 + 512],
                                          zr[32:33, 512:1024])
                        rzb = zp.tile([128, 512], f32, tag="rzb", name="rzb")
                        nc.sync.dma_start(
                            rzb[0:64, :],
                            dz[0:1, o0:o0 + 512].partition_broadcast(64))
                        nc.sync.dma_start(
                            rzb[64:128, :],
                            dz[0:1, o1:o1 + 512].partition_broadcast(64))
                        nc.vector.tensor_mul(
                            uctx[p][:, s0:s0 + 512], uctx[p][:, s0:s0 + 512],
                            rzb[:])

            # ---------- output projection ----------
            with tc.tile_pool(name="ps_o", bufs=2, space="PSUM") as ps_o:
                for ft in range(4):
                    for seg in range(2):
                        po = ps_o.tile([128, 512], f32, tag="po", name="po")
                        for c in range(4):
                            nc.tensor.matmul(
                                po[:],
                                wot[:, c * D + ft * 128:c * D + (ft + 1) * 128],
                                uctx[c][:, seg * 512:(seg + 1) * 512],
                                start=(c == 0), stop=(c == 3))
                        osb = osp.tile([128, 512], f32, tag="osb", name="osb")
                        nc.vector.tensor_copy(osb[:], po[:])
                        nc.sync.dma_start(
                            outT[ft * 128:(ft + 1) * 128,
                                 seg * 512:(seg + 1) * 512], osb[:])
            if rep_ctx is not None:
                rep_ctx.__exit__(None, None, None)
    return nc


_NC_CACHE = None


def kernel(x, W_qkv, W_out):
    global _NC_CACHE
    from concourse.bass_utils import run_bass_kernel_spmd

    x = np.asarray(x, dtype=np.float32)
    W_qkv = np.ascontiguousarray(np.asarray(W_qkv, dtype=np.float32))
    W_out = np.ascontiguousarray(np.asarray(W_out, dtype=np.float32))

    if _NC_CACHE is None:
        _NC_CACHE = _build_nc()
    nc = _NC_CACHE

    xTb = [np.ascontiguousarray(x[b].T) for b in range(B)]
    in_maps = []
    for d in range(NDEV):
        b = d // DEV_PER_BATCH
        r0 = (d % DEV_PER_BATCH) * ROWS
        in_maps.append({
            "xT": xTb[b],
            "xTq": np.ascontiguousarray(xTb[b][:, r0:r0 + ROWS]),
            "wqkv": W_qkv,
            "wout": W_out,
        })

    res = run_bass_kernel_spmd(nc, in_maps, core_ids=list(range(NDEV)))

    out = np.empty((B, S, D), dtype=np.float32)
    for d in range(NDEV):
        b = d // DEV_PER_BATCH
        r0 = (d % DEV_PER_BATCH) * ROWS
        out[b, r0:r0 + ROWS, :] = res.results[d]["outT"].T
    return out
